# revision 3
# baseline (speedup 1.0000x reference)
"""Trainium2 kernel for nn_IonisGateV26: trunk MLP + 9-band MoE heads + gated sidecars.

Strategy (pure data parallel per the sharding hint, plus band routing):
  - Host: sort samples by band, pack into fixed-size single-band segments,
    shard segments across the 8 NeuronCores. Per-segment head weights are
    gathered on host so every core runs the identical program (SPMD).
  - Device: one jitted module per core (pmap over 8 cores). Matmuls run in
    bf16 with fp32 accumulation (TensorE full rate); mish is computed as
    x*(w-1)/(w+1) with w=(1+e^x)^2 — a single-transcendental form, since the
    toolchain has no mish/softplus tables and cannot lower chained
    exp->log->tanh activations.
  - Host: inverse-scatter the routed outputs back to the original order.
"""

import numpy as np
import jax
import jax.numpy as jnp

NC = 8          # NeuronCores
SEG = 1024      # samples per single-band segment

_BF16 = jnp.bfloat16
_F32 = jnp.float32


def _mish(x):
    # x*tanh(softplus(x)) = x*(w-1)/(w+1), w=(1+e^x)^2 — single transcendental
    # (the toolchain cannot lower chained exp->log->tanh activations)
    u = jnp.exp(jnp.minimum(x, 40.0))
    w = (1.0 + u) * (1.0 + u)
    return x * ((w - 1.0) / (w + 1.0))


def _mm(a, w):
    return jnp.matmul(a.astype(_BF16), w.astype(_BF16), preferred_element_type=_F32)


def _core_fn(x_r, hw1s, hb1s, hw2s, hb2s, W):
    """One core's work. x_r: [S*SEG, 17] routed rows; hw1s: [S,256,128] etc."""
    S = hw1s.shape[0]
    xd = x_r[:, :15]
    xs = x_r[:, 15:16]
    xk = x_r[:, 16:17]

    t1 = _mish(_mm(xd, W['tw1']) + W['tb1'])
    t = _mish(_mm(t1, W['tw2']) + W['tb2'])                      # [S*SEG, 256]

    ts = t.reshape(S, SEG, 256)
    hh = _mish(jnp.einsum('sbd,sdh->sbh', ts.astype(_BF16), hw1s.astype(_BF16),
                          preferred_element_type=_F32) + hb1s[:, None, :])
    heads = jnp.einsum('sbh,sh->sb', hh.astype(_BF16), hw2s.astype(_BF16),
                       preferred_element_type=_F32) + hb2s[:, None]
    base = heads.reshape(S * SEG, 1)

    sun_logit = _mm(_mish(_mm(t, W['sw1']) + W['sb1']), W['sw2']) + W['sb2']
    storm_logit = _mm(_mish(_mm(t, W['stw1']) + W['stb1']), W['stw2']) + W['stb2']
    sun_gate = jax.nn.sigmoid(sun_logit)
    storm_gate = jax.nn.sigmoid(storm_logit)

    def mono(v, w1sp, b1, w2sp, b2):
        # w1sp/w2sp already softplus-positivized on host
        h = jnp.tanh(_mm(v, w1sp) + b1)
        return _mm(h, w2sp) + b2

    out = base \
        + sun_gate * mono(xs, W['sun_w1'], W['sun_b1'], W['sun_w2'], W['sun_b2']) \
        + storm_gate * mono(xk, W['storm_w1'], W['storm_b1'], W['storm_w2'], W['storm_b2'])
    return out  # [S*SEG, 1] f32


_PMAP_CACHE = {}


def _get_pmapped(S):
    fn = _PMAP_CACHE.get(S)
    if fn is None:
        fn = jax.pmap(_core_fn, in_axes=(0, 0, 0, 0, 0, None),
                      devices=jax.devices()[:NC])
        _PMAP_CACHE[S] = fn
    return fn


def kernel(**inputs):
    inputs = {k: np.asarray(v) for k, v in inputs.items()}
    x = inputs['x']
    B = x.shape[0]
    band = x[:, 17].astype(np.int32)

    # ---- host routing: sort by band, pack fixed-size single-band segments ----
    order = np.argsort(band, kind='stable')
    counts = np.bincount(band, minlength=9)
    seg_rows = []       # each: (band_k, idx array of length SEG, -1 padded)
    pos = 0
    for k in range(9):
        idx_k = order[pos:pos + counts[k]]
        pos += counts[k]
        for s0 in range(0, len(idx_k), SEG):
            chunk = idx_k[s0:s0 + SEG]
            if len(chunk) < SEG:
                chunk = np.concatenate([chunk, np.full(SEG - len(chunk), -1, np.int64)])
            seg_rows.append((k, chunk))
    n_seg = len(seg_rows)
    S = -(-n_seg // NC)                     # segments per core
    while len(seg_rows) < NC * S:           # dummy all-pad segments
        seg_rows.append((0, np.full(SEG, -1, np.int64)))

    seg_band = np.array([k for k, _ in seg_rows], np.int64)          # [NC*S]
    seg_idx = np.stack([c for _, c in seg_rows])                     # [NC*S, SEG]
    safe_idx = np.where(seg_idx >= 0, seg_idx, 0)

    x_r = x[safe_idx.reshape(-1), :17].reshape(NC, S * SEG, 17).astype(np.float32)
    hw1s = inputs['hw1'][seg_band].reshape(NC, S, 256, 128)
    hb1s = inputs['hb1'][seg_band].reshape(NC, S, 128)
    hw2s = inputs['hw2'][seg_band].reshape(NC, S, 128)
    hb2s = inputs['hb2'][seg_band].reshape(NC, S)

    def _sp(a):  # host softplus (tiny weight tensors)
        a = a.astype(np.float64)
        return (np.maximum(a, 0) + np.log1p(np.exp(-np.abs(a)))).astype(np.float32)

    W = {k: jnp.asarray(inputs[k]) for k in
         ('tw1', 'tb1', 'tw2', 'tb2', 'sw1', 'sb1', 'sw2', 'sb2',
          'stw1', 'stb1', 'stw2', 'stb2',
          'sun_b1', 'sun_b2', 'storm_b1', 'storm_b2')}
    for k in ('sun_w1', 'sun_w2', 'storm_w1', 'storm_w2'):
        W[k] = jnp.asarray(_sp(inputs[k]))

    out_r = _get_pmapped(S)(jnp.asarray(x_r), jnp.asarray(hw1s), jnp.asarray(hb1s),
                            jnp.asarray(hw2s), jnp.asarray(hb2s), W)
    out_r = np.asarray(out_r).reshape(NC * S * SEG)

    # ---- inverse scatter ----
    flat_idx = seg_idx.reshape(-1)
    valid = flat_idx >= 0
    out = np.empty((B, 1), np.float32)
    out[flat_idx[valid], 0] = out_r[valid]
    return out


# revision 5
# speedup vs baseline: 3.4494x; 3.4494x over previous
"""Trainium2 kernel for nn_IonisGateV26: trunk MLP + 9-band MoE heads + gated sidecars.

Strategy (pure data parallel per the sharding hint, plus band routing):
  - Host: sort samples by band, pack into fixed-size single-band segments,
    shard segments across the 8 NeuronCores. Per-segment head weights are
    gathered on host so every core runs the identical program (SPMD).
  - Device: one jitted module per core (pmap over 8 cores). Matmuls run in
    bf16 with fp32 accumulation (TensorE full rate); mish is computed as
    x*(w-1)/(w+1) with w=(1+e^x)^2 — a single-transcendental form, since the
    toolchain has no mish/softplus tables and cannot lower chained
    exp->log->tanh activations.
  - Host: inverse-scatter the routed outputs back to the original order.
"""

import numpy as np
import jax
import jax.numpy as jnp

NC = 8          # NeuronCores
SEG = 1024      # samples per single-band segment

_BF16 = jnp.bfloat16
_F32 = jnp.float32


def _mish(x):
    # x*tanh(softplus(x)) = x*(w-1)/(w+1), w=(1+e^x)^2 — single transcendental
    # (the toolchain cannot lower chained exp->log->tanh activations)
    u = jnp.exp(jnp.minimum(x, 40.0))
    w = (1.0 + u) * (1.0 + u)
    return x * ((w - 1.0) / (w + 1.0))


def _mm(a, w):
    return jnp.matmul(a.astype(_BF16), w.astype(_BF16), preferred_element_type=_F32)


def _core_fn(x_r, seg_band, W):
    """One core's work. x_r: [S*SEG, 17] routed rows (bf16); seg_band: [S] head ids."""
    S = seg_band.shape[0]
    hw1s = W['hw1'][seg_band]        # [S,256,128] gathered on device (9 heads total)
    hb1s = W['hb1'][seg_band]
    hw2s = W['hw2'][seg_band]
    hb2s = W['hb2'][seg_band]
    xd = x_r[:, :15]
    xs = x_r[:, 15:16].astype(_F32)
    xk = x_r[:, 16:17].astype(_F32)

    t1 = _mish(_mm(xd, W['tw1']) + W['tb1'])
    t = _mish(_mm(t1, W['tw2']) + W['tb2'])                      # [S*SEG, 256]

    ts = t.reshape(S, SEG, 256)
    hh = _mish(jnp.einsum('sbd,sdh->sbh', ts.astype(_BF16), hw1s.astype(_BF16),
                          preferred_element_type=_F32) + hb1s[:, None, :])
    heads = jnp.einsum('sbh,sh->sb', hh.astype(_BF16), hw2s.astype(_BF16),
                       preferred_element_type=_F32) + hb2s[:, None]
    base = heads.reshape(S * SEG, 1)

    sun_logit = _mm(_mish(_mm(t, W['sw1']) + W['sb1']), W['sw2']) + W['sb2']
    storm_logit = _mm(_mish(_mm(t, W['stw1']) + W['stb1']), W['stw2']) + W['stb2']
    sun_gate = jax.nn.sigmoid(sun_logit)
    storm_gate = jax.nn.sigmoid(storm_logit)

    def mono(v, w1sp, b1, w2sp, b2):
        # w1sp/w2sp already softplus-positivized on host
        h = jnp.tanh(_mm(v, w1sp) + b1)
        return _mm(h, w2sp) + b2

    out = base \
        + sun_gate * mono(xs, W['sun_w1'], W['sun_b1'], W['sun_w2'], W['sun_b2']) \
        + storm_gate * mono(xk, W['storm_w1'], W['storm_b1'], W['storm_w2'], W['storm_b2'])
    return out  # [S*SEG, 1] f32


_PMAP_CACHE = {}


def _get_pmapped(S):
    fn = _PMAP_CACHE.get(S)
    if fn is None:
        fn = jax.pmap(_core_fn, in_axes=(0, 0, None),
                      devices=jax.devices()[:NC])
        _PMAP_CACHE[S] = fn
    return fn


def kernel(**inputs):
    inputs = {k: np.asarray(v) for k, v in inputs.items()}
    x = inputs['x']
    B = x.shape[0]
    band = x[:, 17].astype(np.int32)

    # ---- host routing: sort by band, pack fixed-size single-band segments ----
    order = np.argsort(band, kind='stable')
    counts = np.bincount(band, minlength=9)
    seg_rows = []       # each: (band_k, idx array of length SEG, -1 padded)
    pos = 0
    for k in range(9):
        idx_k = order[pos:pos + counts[k]]
        pos += counts[k]
        for s0 in range(0, len(idx_k), SEG):
            chunk = idx_k[s0:s0 + SEG]
            if len(chunk) < SEG:
                chunk = np.concatenate([chunk, np.full(SEG - len(chunk), -1, np.int64)])
            seg_rows.append((k, chunk))
    n_seg = len(seg_rows)
    S = -(-n_seg // NC)                     # segments per core
    while len(seg_rows) < NC * S:           # dummy all-pad segments
        seg_rows.append((0, np.full(SEG, -1, np.int64)))

    seg_band = np.array([k for k, _ in seg_rows], np.int64)          # [NC*S]
    seg_idx = np.stack([c for _, c in seg_rows])                     # [NC*S, SEG]
    safe_idx = np.where(seg_idx >= 0, seg_idx, 0)

    import ml_dtypes
    x_r = x[safe_idx.reshape(-1), :17].reshape(NC, S * SEG, 17).astype(ml_dtypes.bfloat16)
    seg_band_c = seg_band.reshape(NC, S).astype(np.int32)

    def _sp(a):  # host softplus (tiny weight tensors)
        a = a.astype(np.float64)
        return (np.maximum(a, 0) + np.log1p(np.exp(-np.abs(a)))).astype(np.float32)

    W = {k: jnp.asarray(inputs[k]) for k in
         ('tw1', 'tb1', 'tw2', 'tb2', 'sw1', 'sb1', 'sw2', 'sb2',
          'stw1', 'stb1', 'stw2', 'stb2',
          'sun_b1', 'sun_b2', 'storm_b1', 'storm_b2',
          'hw1', 'hb1', 'hw2', 'hb2')}
    for k in ('sun_w1', 'sun_w2', 'storm_w1', 'storm_w2'):
        W[k] = jnp.asarray(_sp(inputs[k]))

    out_r = _get_pmapped(S)(jnp.asarray(x_r), jnp.asarray(seg_band_c), W)
    out_r = np.asarray(out_r).reshape(NC * S * SEG)

    # ---- inverse scatter ----
    flat_idx = seg_idx.reshape(-1)
    valid = flat_idx >= 0
    out = np.empty((B, 1), np.float32)
    out[flat_idx[valid], 0] = out_r[valid]
    return out
